# revision 1
# baseline (speedup 1.0000x reference)
"""Concordance-index loss on Trainium2 (8 NeuronCores, Bass/Tile).

Reference math over N=8192 samples (t = exp(event_time), d = event_indicator,
r = estimate), pairwise over ordered pairs (i, j):
    comp(i,j)  = d_i & (t_i < t_j | (t_i == t_j & ~d_j))
    conc       = sum comp & (r_j - r_i < 0)
    tied       = sum comp & |r_j - r_i| <= 1e-8
    total      = sum comp
    disc       = total - conc - tied
    out        = 1 - (disc + 0.5*tied) / (disc + conc + tied + 1e-7)

Device strategy (host does only O(N log N) re-encoding of the three length-N
vectors; all 67M pairwise compares run on the NeuronCores):

 - t is quantized (0.05 grid in log space, and exp is strictly monotone), so
   replace t by its dense rank K_t < 2048.  The predicate
   (t_i < t_j | (t_i == t_j & ~d_j)) collapses to ONE compare
   trank_i < trank_j + 0.5*(1-d_j), and the d_i gate folds in by setting
   censored rows' trank_i to a +32768 sentinel.  All values are fp16-exact.
 - r is replaced by its dense rank, embedded as monotonically increasing fp16
   bit patterns (rank+1024 viewed as fp16) so order compares are exact and the
   DVE can run in 16-bit perf modes.  conc(i,j) = (remb_i > remb_j).
 - The tie band |fl(r_j - r_i)| <= 1e-8f is, for each j, a CONTIGUOUS window
   [lo_j, hi_j] of r-ranks (differences are monotone in the sorted order).
   Windows are found on host with the exact same IEEE f32 subtract the
   reference uses.  tied(i,j) = (remb_i <= hiemb_j) - (remb_i < loemb_j).

Sharding: each of the 8 cores owns a 1024-wide i-slice (free dim) and loops
all 8192 j as 64 partition-chunks of 128.  Per chunk, 4 VectorE instructions
(one tensor_scalar + three scalar_tensor_tensor) compute the masked compares
with inline accum_out row-sums; the host all-reduces the four count tiles and
applies the final scalar formula.
"""

import numpy as np

N = 8192
NCORES = 8
P = 128
IBLK = N // NCORES          # 1024 i's per core (free dim)
CCH = N // P                # 64 j partition-chunks

_CACHE = {}


def _build_nc():
    import concourse.bass as bass
    import concourse.tile as tile
    from concourse import mybir

    dt = mybir.dt
    Alu = mybir.AluOpType

    nc = bass.Bass()
    # All inputs byte-packed into ONE dram tensor (fp16 tm|re broadcast rows,
    # then f32 uj|rj|hij|loj scalars) so the whole kernel uses exactly two
    # DMAs (1 in + 1 out).  More DMA queues -> more semaphores -> the
    # kernel-tail Drain instruction exceeds its tiny ISA sync-wait budget.
    NB16 = 2 * IBLK * 2                 # bytes of fp16 payload per partition
    NB32 = 4 * CCH * 4                  # bytes of f32 payload per partition
    xin = nc.declare_dram_parameter("xin", [P, NB16 + NB32], dt.uint8,
                                    isOutput=False)
    out = nc.declare_dram_parameter("out", [P, 4 * CCH], dt.float32,
                                    isOutput=True)

    # Raw bass (no TileContext): the kernel is straight-line single-engine
    # DVE code, so program order gives all intra-engine dependencies; the
    # only syncs needed are DMA-in -> DVE and DVE -> DMA-out.  (TileContext's
    # tail Drain needs one sync-wait per proc, which exceeds the 2-slot ISA
    # budget of the Drain instruction once a DMA queue is involved.)
    with (
        nc.sbuf_tensor([P, NB16 + NB32], dt.uint8) as xin_s,
        nc.sbuf_tensor([P, 4 * CCH], dt.float32) as out_s,
        nc.sbuf_tensor([P, IBLK], dt.float16) as comp,
        nc.sbuf_tensor([P, IBLK], dt.float16) as dead,
        nc.semaphore() as dsem,
        nc.semaphore() as vsem,
        nc.Block() as block,
    ):
        xf16_s = xin_s[:, 0:NB16].bitcast(dt.float16)
        xf32_s = xin_s[:, NB16:NB16 + NB32].bitcast(dt.float32)
        tmr_s = xf16_s[:, 0:IBLK]
        rke_s = xf16_s[:, IBLK:2 * IBLK]
        uj_s = xf32_s[:, 0 * CCH:1 * CCH]
        rj_s = xf32_s[:, 1 * CCH:2 * CCH]
        hij_s = xf32_s[:, 2 * CCH:3 * CCH]
        loj_s = xf32_s[:, 3 * CCH:4 * CCH]
        cs_s = out_s[:, 0 * CCH:1 * CCH]
        cc_s = out_s[:, 1 * CCH:2 * CCH]
        ta_s = out_s[:, 2 * CCH:3 * CCH]
        tb_s = out_s[:, 3 * CCH:4 * CCH]

        @block.gpsimd
        def _(g):
            g.dma_start(xin_s[:], xin[:]).then_inc(dsem, 16)
            g.wait_ge(vsem, 1)
            g.dma_start(out[:], out_s[:]).then_inc(dsem, 16)

        @block.vector
        def _(v):
            v.wait_ge(dsem, 16)
            last = None
            for jc in range(CCH):
                col = slice(jc, jc + 1)
                v.tensor_scalar(
                    comp[:], tmr_s, uj_s[:, col], None, Alu.is_lt,
                    op1=Alu.add,  # with accum_out, op1 is the reduce op
                    accum_out=cs_s[:, col],
                )
                v.scalar_tensor_tensor(
                    dead[:], rke_s, rj_s[:, col], comp[:],
                    op0=Alu.is_gt, op1=Alu.mult, accum_out=cc_s[:, col],
                )
                v.scalar_tensor_tensor(
                    dead[:], rke_s, hij_s[:, col], comp[:],
                    op0=Alu.is_le, op1=Alu.mult, accum_out=ta_s[:, col],
                )
                last = v.scalar_tensor_tensor(
                    dead[:], rke_s, loj_s[:, col], comp[:],
                    op0=Alu.is_lt, op1=Alu.mult, accum_out=tb_s[:, col],
                )
            last.then_inc(vsem, 1)

    return nc


def _prep_inputs(event_indicator, event_time, estimate):
    d = np.asarray(event_indicator).reshape(-1).astype(bool)
    t = np.asarray(event_time, dtype=np.float32).reshape(-1)
    r = np.asarray(estimate, dtype=np.float32).reshape(-1)
    n = t.shape[0]
    assert n == N

    # t dense ranks.  exp is strictly increasing and injective on the
    # reference's 0.05-grid log-times, so ranking the raw times preserves
    # both the order and the equality structure of t = exp(event_time).
    tv = np.unique(t)
    trk = np.searchsorted(tv, t).astype(np.float32)
    assert len(tv) + 1.0 < 2040.0, "t ranks must stay fp16-exact"
    u = (trk + np.float32(0.5) * (~d).astype(np.float32)).astype(np.float16)
    tm = np.where(d, trk, np.float32(32768.0)).astype(np.float16)

    # r dense ranks -> monotone fp16 embedding (normal range, no denormals).
    rv = np.unique(r)
    m = len(rv)
    assert m + 1024 < 31744, "r rank embedding must stay in normal fp16 range"
    emb = (np.arange(m, dtype=np.uint16) + np.uint16(1024)).view(np.float16)
    rrk = np.searchsorted(rv, r)

    # Tie windows: for each unique value k, the set of m with
    # |fl(rv[m] - rv[k])| <= 1e-8f is contiguous; two-pointer sweep using
    # the same IEEE f32 subtract as the reference's rdiff.
    thr = np.float32(1e-8)
    lo = np.zeros(m, dtype=np.int64)
    hi = np.zeros(m, dtype=np.int64)
    p = 0
    for k in range(m):
        while np.abs(rv[k] - rv[p]) > thr:
            p += 1
        lo[k] = p
    p = m - 1
    for k in range(m - 1, -1, -1):
        while np.abs(rv[k] - rv[p]) > thr:
            p -= 1
        hi[k] = p

    r_e = emb[rrk]
    lo_e = emb[lo[rrk]]
    hi_e = emb[hi[rrk]]

    def jscalar(x):
        # j = jc*128 + p  ->  element [p, jc] of a [128, 64] tile
        # (f32: compare-op scalar operands must be float32; fp16->f32 is exact)
        return np.ascontiguousarray(x.reshape(CCH, P).T.astype(np.float32))

    uj = jscalar(u)
    rj = jscalar(r_e)
    hij = jscalar(hi_e)
    loj = jscalar(lo_e)

    xf32 = np.ascontiguousarray(np.concatenate([uj, rj, hij, loj], axis=1))
    b32 = xf32.view(np.uint8).reshape(P, -1)
    in_maps = []
    for c in range(NCORES):
        blk = slice(c * IBLK, (c + 1) * IBLK)
        row16 = np.concatenate([tm[blk], r_e[blk]])
        b16 = np.ascontiguousarray(
            np.broadcast_to(row16[None, :], (P, 2 * IBLK))).view(np.uint8)
        in_maps.append({
            "xin": np.ascontiguousarray(np.concatenate([b16, b32], axis=1)),
        })
    return in_maps


def _finish(results):
    tot = np.float64(0.0)
    conc = np.float64(0.0)
    tie_a = np.float64(0.0)
    tie_b = np.float64(0.0)
    for res in results:
        o = res["out"].astype(np.float64)
        tot += o[:, 0 * CCH:1 * CCH].sum()
        conc += o[:, 1 * CCH:2 * CCH].sum()
        tie_a += o[:, 2 * CCH:3 * CCH].sum()
        tie_b += o[:, 3 * CCH:4 * CCH].sum()
    tied = tie_a - tie_b
    disc = tot - conc - tied
    loss = (disc + 0.5 * tied) / (disc + conc + tied + 1e-7)
    return np.asarray(1.0 - loss, dtype=np.float32)


def kernel(event_indicator, event_time, estimate):
    from concourse.bass_utils import run_bass_kernel_spmd

    in_maps = _prep_inputs(event_indicator, event_time, estimate)
    if "nc" not in _CACHE:
        _CACHE["nc"] = _build_nc()
    nc = _CACHE["nc"]
    out = run_bass_kernel_spmd(nc, in_maps, core_ids=list(range(NCORES)))
    return _finish(out.results)



# revision 2
# speedup vs baseline: 20.5491x; 20.5491x over previous
"""Concordance-index loss on Trainium2 (8 NeuronCores, raw Bass) — v5.

Same math/encoding as v2/v4, but the layout is FLIPPED (i on partitions,
j on the free dim) so all three compute engines work in parallel:

  DVE,  per i-tile t of 16:   comp = (u_j > tm_i)        # ts, 4x, ~0.9us
                              rgt  = (r_j < re_i)        # ts, 4x, ~0.9us
                              prod = min(comp, rgt)      # tt, 2x, ~2.0us
  ActE, 1 tile behind:        total_col[t] += sum_j comp     # Copy+accum
  PE,   1 tile behind:        conc_psum[j] += sum_i prod     # ones-matmul,
                                 8 N=512 matmuls, PSUM accumulates over t

The global `total` only needs a scalar, so Act's per-partition (per-i) free
dim reduction is fine; `conc` per-j partials accumulate in PSUM partition 0
([1, 4096] f32 = exactly the 16 KiB PSUM of one partition) and are DMA'd
out at the end.  comp/prod tiles ping-pong as flat tensors (3-D slice APs
measured ~2x slower on the ts) with two ack semaphores for back-pressure.

Sharding: 8 cores = 4 i-quarters (2048 i's = 16 partition-tiles of 128) x
2 j-halves (4096 j's, free dim).  `tied` is enumerated exactly on host as
in v2 (measure-zero band); host sums the count tiles and applies the final
scalar formula.
"""

import numpy as np

N = 8192
NCORES = 8
P = 128
NIQ = 4                     # i-quarters
NJH = 2                     # j-halves
IBLK = N // NIQ             # 2048 i's per core
IT = IBLK // P              # 16 i partition-tiles per core
JW = N // NJH               # 4096 j's per core (free dim)
MMW = 512                   # max moving free dim per matmul
NB16 = (2 * JW + 2) * 2     # fp16 payload/partition: u|r rows + ones + pad
NB32 = 2 * IT * 4           # f32 payload/partition: tmi|rei scalars
NOUT = IT                   # f32 outputs per partition in "out" (total)

_CACHE = {}


def _build_nc(repeat=1, pe_mode="full"):
    # repeat>1 wraps the compute pass in a hardware Fori loop on all
    # engines; used only for slope-based timing (R>1 outputs are NOT valid
    # -- the ping-pong handshake thresholds are per-iteration).
    import concourse.bass as bass
    from concourse import mybir

    dt = mybir.dt
    Alu = mybir.AluOpType
    Act = mybir.ActivationFunctionType

    nc = bass.Bass()
    xin = nc.declare_dram_parameter("xin", [P, NB16 + NB32], dt.uint8,
                                    isOutput=False)
    out = nc.declare_dram_parameter("out", [P, NOUT], dt.float32,
                                    isOutput=True)
    out2 = nc.declare_dram_parameter("out2", [1, JW], dt.float32,
                                     isOutput=True)

    with (
        nc.sbuf_tensor([P, NB16 + NB32], dt.uint8) as xin_s,
        nc.sbuf_tensor([P, NOUT], dt.float32) as out_s,
        nc.sbuf_tensor([P, JW], dt.float16) as comp0,
        nc.sbuf_tensor([P, JW], dt.float16) as comp1,
        nc.sbuf_tensor([P, JW], dt.float16) as prod0,
        nc.sbuf_tensor([P, JW], dt.float16) as prod1,
        nc.sbuf_tensor([P, JW], dt.float16) as rgt,
        nc.sbuf_tensor([P, JW], dt.float16) as deadA,
        nc.sbuf_tensor([1, JW], dt.float32) as out2_s,
        nc.psum_tensor([P, JW], dt.float32) as psumT,
        nc.semaphore() as dsem,
        nc.semaphore() as csemA,
        nc.semaphore() as csemP,
        nc.semaphore() as asem,
        nc.semaphore() as psem,
        nc.semaphore() as vsem,
        nc.Block() as block,
    ):
        xf16_s = xin_s[:, 0:NB16].bitcast(dt.float16)
        xf32_s = xin_s[:, NB16:NB16 + NB32].bitcast(dt.float32)
        uj_row = xf16_s[:, 0:JW]
        rj_row = xf16_s[:, JW:2 * JW]
        ones_w = xf16_s[:, 2 * JW:2 * JW + 1]     # [128, 1] of 1.0
        tmi_s = xf32_s[:, 0 * IT:1 * IT]
        rei_s = xf32_s[:, 1 * IT:2 * IT]
        comps = [comp0, comp1]
        prods = [prod0, prod1]

        @block.gpsimd
        def _(g):
            g.dma_start(xin_s[:], xin[:]).then_inc(dsem, 16)
            g.wait_ge(vsem, 3)
            g.dma_start(out[:], out_s[:]).then_inc(dsem, 16)
            g.dma_start(out2[:], out2_s[:]).then_inc(dsem, 16)

        @block.vector
        def _(v):
            v.wait_ge(dsem, 16)

            def one_pass():
                for t in range(IT):
                    col = slice(t, t + 1)
                    if t >= 2:
                        v.wait_ge(asem, t - 1)   # Act freed comp slot
                        v.wait_ge(psem, t - 1)   # PE freed prod slot
                    # comp = (u_j > tm_i)  [4x]
                    v.tensor_scalar(
                        comps[t % 2][:], uj_row, tmi_s[:, col], None,
                        Alu.is_gt).then_inc(csemA, 1)
                    # rgt = (r_j < re_i)  [4x]
                    v.tensor_scalar(
                        rgt[:], rj_row, rei_s[:, col], None, Alu.is_lt)
                    # prod = comp & rgt  [2x]
                    v.tensor_tensor(
                        prods[t % 2][:], comps[t % 2][:], rgt[:],
                        Alu.min).then_inc(csemP, 1)

            if repeat == 1:
                one_pass()
            else:
                with v.Fori(0, repeat) as _i:
                    one_pass()
            # evacuate PE's conc partials (DMA cannot read PSUM)
            v.wait_ge(psem, IT * repeat)
            v.tensor_copy(out2_s[:], psumT[0:1, :]).then_inc(vsem, 1)

        @block.scalar
        def _(s):
            def one_pass():
                for t in range(IT):
                    s.wait_ge(csemA, t + 1)
                    # total_col[t] = sum_j comp  [Act engine]
                    s.activation(deadA[:], comps[t % 2][:], Act.Copy,
                                 accum_out=out_s[:, t:t + 1])
                    s.nop().then_inc(asem, 1)

            if repeat == 1:
                one_pass()
            else:
                with s.Fori(0, repeat) as _i:
                    one_pass()
            s.memzero(deadA[:, 0:2])
            s.nop().then_inc(vsem, 1)

        @block.tensor
        def _(te):
            te.wait_ge(dsem, 16)

            def one_pass():
                for t in range(IT):
                    te.wait_ge(csemP, t + 1)
                    if pe_mode == "none":
                        te.nop().then_inc(psem, 1)
                        continue
                    nmm = 1 if pe_mode == "one" else JW // MMW
                    last = None
                    for m in range(nmm):
                        sl = slice(m * MMW, (m + 1) * MMW)
                        # conc_psum[j] += sum_i prod[i, j]
                        last = te.matmul(
                            psumT[0:1, sl], ones_w, prods[t % 2][:, sl],
                            start=(t == 0), stop=(t == IT - 1))
                    last.then_inc(psem, 1)

            if repeat == 1:
                one_pass()
            else:
                with te.Fori(0, repeat) as _i:
                    one_pass()
            te.nop().then_inc(vsem, 1)

    return nc


def _encode(event_indicator, event_time, estimate):
    d = np.asarray(event_indicator).reshape(-1).astype(bool)
    t = np.asarray(event_time, dtype=np.float32).reshape(-1)
    r = np.asarray(estimate, dtype=np.float32).reshape(-1)
    assert t.shape[0] == N

    tv = np.unique(t)
    trk = np.searchsorted(tv, t).astype(np.float32)
    # fp16 must represent trk and trk+0.5 exactly -> need trk+1 < 1024
    assert len(tv) + 2 < 1024, "t ranks must stay fp16-exact incl. +0.5"
    return d, t, r, trk


def _prep_inputs(event_indicator, event_time, estimate):
    d, _t, r, trk = _encode(event_indicator, event_time, estimate)

    u = (trk + np.float32(0.5) * (~d).astype(np.float32)).astype(np.float16)
    tm = np.where(d, trk, np.float32(32768.0)).astype(np.float16)

    rv = np.unique(r)
    m = len(rv)
    assert m + 1024 < 31744, "r rank embedding must stay in normal fp16 range"
    emb = (np.arange(m, dtype=np.uint16) + np.uint16(1024)).view(np.float16)
    r_e = emb[np.searchsorted(rv, r)]

    in_maps = []
    for c in range(NCORES):
        iq, jh = divmod(c, NJH)
        isl = slice(iq * IBLK, (iq + 1) * IBLK)
        jsl = slice(jh * JW, (jh + 1) * JW)
        # i = iq*IBLK + t*128 + p  ->  element [p, t] of a [128, IT] tile
        tmi = np.ascontiguousarray(
            tm[isl].astype(np.float32).reshape(IT, P).T)
        rei = np.ascontiguousarray(
            r_e[isl].astype(np.float32).reshape(IT, P).T)
        b32 = np.ascontiguousarray(
            np.concatenate([tmi, rei], axis=1)).view(np.uint8).reshape(P, -1)
        row16 = np.concatenate([
            u[jsl], r_e[jsl],
            np.array([1.0, 0.0], dtype=np.float16),   # ones weight + pad
        ])
        b16 = np.ascontiguousarray(
            np.broadcast_to(row16[None, :], (P, 2 * JW + 2))).view(np.uint8)
        in_maps.append({
            "xin": np.ascontiguousarray(np.concatenate([b16, b32], axis=1)),
        })
    return in_maps


def _tied_host(event_indicator, event_time, estimate):
    """Exact tied_risk count: sum over ordered pairs (i,j) of
    comp(i,j) & (|f32(r_j - r_i)| <= 1e-8), enumerating only the near-tied
    band via a two-pointer sweep over sorted r (same IEEE f32 subtract as
    the reference's rdiff; O(N log N + T) with T ~ handful for f32 data)."""
    d, _t, r, trk = _encode(event_indicator, event_time, estimate)

    thr = np.float32(1e-8)
    order = np.argsort(r, kind="stable")
    rs = r[order]
    lo = np.zeros(N, dtype=np.int64)
    hi = np.zeros(N, dtype=np.int64)
    p = 0
    for k in range(N):
        while np.abs(rs[k] - rs[p]) > thr:
            p += 1
        lo[k] = p
    p = N - 1
    for k in range(N - 1, -1, -1):
        while np.abs(rs[k] - rs[p]) > thr:
            p -= 1
        hi[k] = p

    cnt = hi - lo + 1
    T = int(cnt.sum())
    K = np.repeat(np.arange(N, dtype=np.int64), cnt)
    offs = np.concatenate(([0], np.cumsum(cnt)[:-1]))
    Ppos = np.arange(T, dtype=np.int64) - np.repeat(offs, cnt) + np.repeat(lo, cnt)
    i_idx = order[K]       # row sample (needs event)
    j_idx = order[Ppos]    # column sample
    comp = d[i_idx] & (
        (trk[i_idx] < trk[j_idx])
        | ((trk[i_idx] == trk[j_idx]) & (~d[j_idx]))
    )
    return float(comp.sum())


def _finish(results, tied):
    tot = np.float64(0.0)
    conc = np.float64(0.0)
    for res in results:
        tot += res["out"].astype(np.float64).sum()
        conc += res["out2"].astype(np.float64).sum()
    disc = tot - conc - tied
    loss = (disc + 0.5 * tied) / (disc + conc + tied + 1e-7)
    return np.asarray(1.0 - loss, dtype=np.float32)


def kernel(event_indicator, event_time, estimate):
    from concourse.bass_utils import run_bass_kernel_spmd

    in_maps = _prep_inputs(event_indicator, event_time, estimate)
    tied = _tied_host(event_indicator, event_time, estimate)
    if "nc" not in _CACHE:
        _CACHE["nc"] = _build_nc()
    nc = _CACHE["nc"]
    out = run_bass_kernel_spmd(nc, in_maps, core_ids=list(range(NCORES)))
    return _finish(out.results, tied)


# revision 3
# speedup vs baseline: 29.6056x; 1.4407x over previous
"""Concordance-index loss on Trainium2 (8 NeuronCores, raw Bass) — v6.

Staircase decomposition on top of v5's flipped layout (i on partitions, j on
the free dim; DVE compares, PE ones-matmul reduction into PSUM):

Sort i by tm and j by u on host.  comp[:, j] = (u_j > tm_i) is then a
prefix (of length p_j) of the sorted-i order, so for each j-column and each
i-tile the tile is all-zeros (PE skips it), all-ones (conc contribution =
sum_i rgt[i, j]: PE reduces rgt directly, no mask computed), or the ONE
boundary tile, where comp and comp&rgt are computed elementwise over a
narrow contiguous j-window.  The DVE computes the full rgt compare (4x)
plus the small boundary window; the PE accumulates into PSUM.  `total` is
exact host rank math (sum_j p_j) and `tied` is host-enumerated as in v2 --
`conc`, the only genuinely 2-D O(N^2) reduction, stays fully on-device.

SPMD needs ONE program for all 8 cores, but the window boundaries are
data-dependent, so the cores take STRIDED shards: core c = (iq, jh) owns
sorted-i positions t*512 + 4*p + iq (16 tiles of 128) and sorted-j
positions 2*k + jh (4096 columns).  Every core's tile t then spans the same
global-i band [t*512, (t+1)*512), making the per-tile windows near
identical across cores; the compiled program uses their union (correct for
every core: outside its own window a column is genuinely all-ones/zeros,
and the elementwise path is always correct).  The program is built (and
cached) per window structure.
"""

from contextlib import ExitStack

import numpy as np

N = 8192
NCORES = 8
P = 128
NIQ = 4                     # i stride (quarters)
NJH = 2                     # j stride (halves)
IBLK = N // NIQ             # 2048 i's per core
IT = IBLK // P              # 16 i partition-tiles per core
IBAND = P * NIQ             # 512: global sorted-i band per tile
JW = N // NJH               # 4096 j's per core (free dim)
MMW = 512                   # max moving free dim per matmul
NB16 = (2 * JW + 4) * 2     # fp16/partition: u|r rows + ones|zero|pad
NB32 = 2 * IT * 4           # f32 payload/partition: tmi|rei scalars
NOUT = 2                    # dummy [P, 2] f32 main out (sums come via out2)

_CACHE = {}


def _mm_slices(lo, hi):
    out = []
    while lo < hi:
        w = min(MMW, hi - lo)
        out.append((lo, lo + w))
        lo += w
    return out


def _build_nc(windows, repeat=1):
    """windows: per tile t, (lo_t, hi_t): columns [0, lo_t) are all-zeros,
    [lo_t, hi_t) boundary (elementwise), [hi_t, JW) all-ones."""
    import concourse.bass as bass
    from concourse import mybir

    dt = mybir.dt
    Alu = mybir.AluOpType

    nc = bass.Bass()
    xin = nc.declare_dram_parameter("xin", [P, NB16 + NB32], dt.uint8,
                                    isOutput=False)
    out = nc.declare_dram_parameter("out", [P, NOUT], dt.float32,
                                    isOutput=True)
    out2 = nc.declare_dram_parameter("out2", [1, JW], dt.float32,
                                     isOutput=True)

    with (
        nc.sbuf_tensor([P, NB16 + NB32], dt.uint8) as xin_s,
        nc.sbuf_tensor([P, NOUT], dt.float32) as out_s,
        nc.sbuf_tensor([P, JW], dt.float16) as rgt0,
        nc.sbuf_tensor([P, JW], dt.float16) as rgt1,
        nc.sbuf_tensor([P, JW], dt.float16) as comp,
        nc.sbuf_tensor([P, JW], dt.float16) as prod0,
        nc.sbuf_tensor([P, JW], dt.float16) as prod1,
        nc.sbuf_tensor([1, JW], dt.float32) as out2_s,
        nc.psum_tensor([P, JW], dt.float32) as psumT,
        nc.semaphore() as dsem,
        nc.semaphore() as csemP,
        nc.semaphore() as psem,
        nc.semaphore() as vsem,
        nc.Block() as block,
    ):
        xf16_s = xin_s[:, 0:NB16].bitcast(dt.float16)
        xf32_s = xin_s[:, NB16:NB16 + NB32].bitcast(dt.float32)
        uj_row = xf16_s[:, 0:JW]
        rj_row = xf16_s[:, JW:2 * JW]
        ones_w = xf16_s[:, 2 * JW:2 * JW + 1]       # [128, 1] of 1.0
        zero_w = xf16_s[:, 2 * JW + 1:2 * JW + 2]   # [128, 1] of 0.0
        tmi_s = xf32_s[:, 0 * IT:1 * IT]
        rei_s = xf32_s[:, 1 * IT:2 * IT]
        rgts = [rgt0, rgt1]
        prods = [prod0, prod1]

        @block.gpsimd
        def _(g):
            g.dma_start(xin_s[:], xin[:]).then_inc(dsem, 16)
            g.wait_ge(vsem, 1)
            g.dma_start(out[:], out_s[:]).then_inc(dsem, 16)
            g.dma_start(out2[:], out2_s[:]).then_inc(dsem, 16)

        @block.vector
        def _(v):
            v.wait_ge(dsem, 16)

            def one_pass():
                for t in range(IT):
                    col = slice(t, t + 1)
                    lo, hi = windows[t]
                    if t >= 2:
                        v.wait_ge(psem, t)       # PE freed this slot pair
                    # rgt = (r_j < re_i)  [4x] -- full row
                    v.tensor_scalar(
                        rgts[t % 2][:], rj_row, rei_s[:, col], None,
                        Alu.is_lt)
                    if hi > lo:
                        # boundary window: elementwise comp and AND
                        v.tensor_scalar(
                            comp[:, lo:hi], uj_row[:, lo:hi], tmi_s[:, col],
                            None, Alu.is_gt)
                        v.tensor_tensor(
                            prods[t % 2][:, lo:hi], comp[:, lo:hi],
                            rgts[t % 2][:, lo:hi], Alu.min)
                    # drain-then-inc: make the tile's SBUF writes visible
                    # before the PE wakes on csemP
                    v.drain()
                    v.sem_inc(csemP, 1)

            if repeat == 1:
                one_pass()
            else:
                with v.Fori(0, repeat) as _i:
                    one_pass()
            # evacuate PE's conc partials (DMA cannot read PSUM)
            v.wait_ge(psem, (IT + 2) * repeat)
            v.memset(out_s[:], 0.0)
            v.tensor_copy(out2_s[:], psumT[0:1, :])
            v.drain()
            v.sem_inc(vsem, 1)

        @block.tensor
        def _(te):
            te.wait_ge(dsem, 16)

            def one_pass():
                # zero all psum cols (zero weights x finite u-row data)
                last = None
                for mlo, mhi in _mm_slices(0, JW):
                    last = te.matmul(
                        psumT[0:1, mlo:mhi], zero_w, uj_row[:, mlo:mhi],
                        start=True, stop=False, skip_group_check=True)
                last.then_inc(psem, 1)
                for t in range(IT):
                    te.wait_ge(csemP, t + 1)
                    lo, hi = windows[t]
                    last = None
                    # boundary: conc += sum_i comp & rgt
                    for mlo, mhi in _mm_slices(lo, hi):
                        last = te.matmul(
                            psumT[0:1, mlo:mhi], ones_w,
                            prods[t % 2][:, mlo:mhi],
                            start=False, stop=False, skip_group_check=True)
                    # all-ones region: conc += sum_i rgt  (j >= hi)
                    for mlo, mhi in _mm_slices(hi, JW):
                        last = te.matmul(
                            psumT[0:1, mlo:mhi], ones_w,
                            rgts[t % 2][:, mlo:mhi],
                            start=False, stop=False, skip_group_check=True)
                    if last is None:
                        te.nop().then_inc(psem, 1)
                    else:
                        last.then_inc(psem, 1)
                # flush the systolic array: MMs retire before their columns
                # finish draining into PSUM, and the DVE evacuation copy
                # must not race the in-flight writes of the final matmuls
                te.drain()
                te.nop().then_inc(psem, 1)

            if repeat == 1:
                one_pass()
            else:
                with te.Fori(0, repeat) as _i:
                    one_pass()

    return nc


def _encode(event_indicator, event_time, estimate):
    d = np.asarray(event_indicator).reshape(-1).astype(bool)
    t = np.asarray(event_time, dtype=np.float32).reshape(-1)
    r = np.asarray(estimate, dtype=np.float32).reshape(-1)
    assert t.shape[0] == N

    tv = np.unique(t)
    trk = np.searchsorted(tv, t).astype(np.float32)
    # fp16 must represent trk and trk+0.5 exactly -> need trk+1 < 1024
    assert len(tv) + 2 < 1024, "t ranks must stay fp16-exact incl. +0.5"
    return d, t, r, trk


def _structure(event_indicator, event_time, estimate):
    """Sorted orders, encodings, exact total, and per-tile union windows."""
    d, _t, r, trk = _encode(event_indicator, event_time, estimate)

    u = (trk + np.float32(0.5) * (~d).astype(np.float32)).astype(np.float16)
    tm = np.where(d, trk, np.float32(32768.0)).astype(np.float16)

    rv = np.unique(r)
    m = len(rv)
    assert m + 1024 < 31744, "r rank embedding must stay in normal fp16 range"
    emb = (np.arange(m, dtype=np.uint16) + np.uint16(1024)).view(np.float16)
    r_e = emb[np.searchsorted(rv, r)]

    iord = np.argsort(tm.astype(np.float32), kind="stable")  # i by tm asc
    jord = np.argsort(u.astype(np.float32), kind="stable")   # j by u asc
    tms = tm[iord].astype(np.float32)
    us = u[jord].astype(np.float32)

    # p_j = #{i: tm_i < u_j}: prefix length in sorted-i order (exact ints)
    pj_sorted = np.searchsorted(tms, us, side="left")
    total = float(pj_sorted.sum())

    # union windows over cores: core (iq, jh) takes j positions 2k+jh; its
    # tile t spans global-i band [t*512, (t+1)*512).  Column k is all-zeros
    # for tile t iff pj <= t*512, all-ones iff pj >= (t+1)*512.
    windows = []
    for t in range(IT):
        lo_u, hi_u = JW, 0
        for jh in range(NJH):
            pj_loc = pj_sorted[jh::NJH]
            lo = int(np.searchsorted(pj_loc, t * IBAND, side="right"))
            hi = int(np.searchsorted(pj_loc, (t + 1) * IBAND, side="left"))
            lo_u, hi_u = min(lo_u, lo), max(hi_u, hi)
        lo_u &= ~1   # keep 4B alignment for the fp16 window ops
        windows.append((lo_u, hi_u))
    return d, r, trk, u, tm, r_e, iord, jord, total, tuple(windows)


def _prep_inputs(event_indicator, event_time, estimate):
    (_d, _r, _trk, u, tm, r_e, iord, jord, _total,
     _windows) = _structure(event_indicator, event_time, estimate)

    in_maps = []
    for c in range(NCORES):
        iq, jh = divmod(c, NJH)
        li = np.arange(IBLK)
        isel = iord[(li // P) * IBAND + (li % P) * NIQ + iq]
        jsel = jord[jh::NJH]
        tmi = np.ascontiguousarray(
            tm[isel].astype(np.float32).reshape(IT, P).T)
        rei = np.ascontiguousarray(
            r_e[isel].astype(np.float32).reshape(IT, P).T)
        b32 = np.ascontiguousarray(
            np.concatenate([tmi, rei], axis=1)).view(np.uint8).reshape(P, -1)
        row16 = np.concatenate([
            u[jsel], r_e[jsel],
            np.array([1.0, 0.0, 0.0, 0.0], dtype=np.float16),
        ])
        b16 = np.ascontiguousarray(
            np.broadcast_to(row16[None, :], (P, 2 * JW + 4))).view(np.uint8)
        in_maps.append({
            "xin": np.ascontiguousarray(np.concatenate([b16, b32], axis=1)),
        })
    return in_maps


def _tied_host(event_indicator, event_time, estimate):
    """Exact tied_risk count (see kernel_v2 docstring)."""
    d, _t, r, trk = _encode(event_indicator, event_time, estimate)

    thr = np.float32(1e-8)
    order = np.argsort(r, kind="stable")
    rs = r[order]
    lo = np.zeros(N, dtype=np.int64)
    hi = np.zeros(N, dtype=np.int64)
    p = 0
    for k in range(N):
        while np.abs(rs[k] - rs[p]) > thr:
            p += 1
        lo[k] = p
    p = N - 1
    for k in range(N - 1, -1, -1):
        while np.abs(rs[k] - rs[p]) > thr:
            p -= 1
        hi[k] = p

    cnt = hi - lo + 1
    T = int(cnt.sum())
    K = np.repeat(np.arange(N, dtype=np.int64), cnt)
    offs = np.concatenate(([0], np.cumsum(cnt)[:-1]))
    Ppos = np.arange(T, dtype=np.int64) - np.repeat(offs, cnt) + np.repeat(lo, cnt)
    i_idx = order[K]
    j_idx = order[Ppos]
    comp = d[i_idx] & (
        (trk[i_idx] < trk[j_idx])
        | ((trk[i_idx] == trk[j_idx]) & (~d[j_idx]))
    )
    return float(comp.sum())


def _finish(results, total, tied):
    conc = np.float64(0.0)
    for res in results:
        conc += res["out2"].astype(np.float64).sum()
    disc = total - conc - tied
    loss = (disc + 0.5 * tied) / (disc + conc + tied + 1e-7)
    return np.asarray(1.0 - loss, dtype=np.float32)


def kernel(event_indicator, event_time, estimate):
    from concourse.bass_utils import run_bass_kernel_spmd

    st = _structure(event_indicator, event_time, estimate)
    total, windows = st[8], st[9]
    in_maps = _prep_inputs(event_indicator, event_time, estimate)
    tied = _tied_host(event_indicator, event_time, estimate)

    if _CACHE.get("windows") != windows:
        _CACHE["nc"] = _build_nc(windows)
        _CACHE["windows"] = windows
        _CACHE["primed"] = False
    nc = _CACHE["nc"]
    # Priming run: on the literal first execution after device load, the
    # PSUM zero-pass write of the final matmul slice does not take effect
    # (boot-state PSUM garbage survives under the accumulation for the last
    # ~46 columns); every execution >= 2 is exact.  Execute twice and use
    # the steady-state result.
    if not _CACHE.get("primed"):
        run_bass_kernel_spmd(nc, in_maps, core_ids=list(range(NCORES)))
        _CACHE["primed"] = True
    out = run_bass_kernel_spmd(nc, in_maps, core_ids=list(range(NCORES)))
    return _finish(out.results, total, tied)
